# revision 8
# baseline (speedup 1.0000x reference)
"""DCNv2 (deformable conv) Trainium2 Bass kernel, v2.

Strategy (per core, pure batch data-parallel across 8 cores):
  - x padded (+1) on host, cast bf16; streamed per band of RB=8 output rows.
  - PE computes offset/mask 3x3 convs (9 accumulating matmuls per output row)
    and per-tap 1x1 convs YT[s, r, k, o] = sum_c W[o,c,k] x(c,r,s) into a
    banded SBUF tensor (bf16), rows r with +-3 halo.
  - DVE builds per-pixel bilinear interpolation fields vy (y-weights) and
    hxm (x-weights with mask folded in), both [w, (hh, k, 5)] bf16.
  - The x-axis interpolation (column gather + lerp, a 7-banded per-(h,k)
    matrix Ex[s, w]) runs on PE: Ex is materialized densely via a DRAM
    round trip — hxm is partition-shifted into a band tensor HB[s,(hh,k,u)]
    (7 SBUF->SBUF DMA copies), HB is scattered to a DRAM image with a
    135-vs-134 skewed stride so the 7-wide band lands on the diagonals of
    dense 128x134 blocks (rest stays pre-zeroed), then read back dense.
    One matmul per (h, k): psum[w, (o, ty)] = Ex^T @ YT[:, rows(ty), k, o].
  - The y-axis interpolation is elementwise: multiply by vy (broadcast over
    o) and reduce over the 5 ty candidates -> red[w, o] bf16; spread across
    DVE / GpSimd / ACT per a static tap assignment.
  - Per-tap results accumulate on PE via transposing matmuls with an
    identity rhs: psum[o, w] += red_k^T; ACT adds bias on the PSUM->SBUF
    copy; DMA out per band.
"""

import sys

sys.path.insert(0, "/opt/trn_rl_repo")

import numpy as np

import concourse.bacc as bacc
import concourse.bass as bass
import concourse.mybir as mybir
from concourse.tile import TileContext

F32 = mybir.dt.float32
BF16 = mybir.dt.bfloat16
AF = mybir.ActivationFunctionType
AL = mybir.AluOpType

C = 96
O = 96
NTAP = 9
W = 128
NCORES = 8
NTY = 5                     # ty candidates (floor(dy) in [-2,1] + 1)
ROWL = 134                  # dense Ex row length (w' = w + 3, padded)
BLKSZ = 128 * ROWL          # 17152 elements per (hh,k) block

# per-tap engine assignment for the y-combine multiply:
#   "direct"   : DVE mult straight from PSUM (fp32 read)
#   "act_dve"  : ACT copies PSUM->SBUF bf16, DVE mult (2x mode)
#   "act_pool" : ACT copies PSUM->SBUF bf16, GpSimd mult
# (the ty-reduction always runs on DVE; GpSimd cannot reduce free axes)
TAPMODE = ["act_dve", "act_pool", "direct", "act_dve", "act_pool",
           "direct", "act_dve", "act_pool", "act_dve"]


def build_nc(H=128, BS=2, RB=8, num_devices=NCORES):
    RS = NTAP * O           # YT row stride = 864
    NROW = RB + 6           # YT band rows incl +-3 halo
    NK9 = RB * NTAP
    BLKS = RB * NTAP        # dense Ex blocks per band = 72
    Hp, Wp = H + 2, W + 2
    XBROW = RB + 6
    FVY = NK9 * NTY         # 360
    FHB = NK9 * 7           # 504
    FEX = NK9 * W           # 9216
    assert H % RB == 0
    nbands = H // RB

    nc = bacc.Bacc("TRN2", target_bir_lowering=False, debug=False,
                   num_devices=num_devices, dynamic_dma_scratch_size=2048)

    xp = nc.dram_tensor("xp", [BS, C, Hp * Wp], BF16, kind="ExternalInput")
    wmain = nc.dram_tensor("wmain", [C, NTAP * O], BF16, kind="ExternalInput")
    womb = nc.dram_tensor("womb", [C, NTAP * 27], BF16, kind="ExternalInput")
    obrep = nc.dram_tensor("obrep", [W, 27], F32, kind="ExternalInput")
    ity = nc.dram_tensor("ity", [W, NTY], F32, kind="ExternalInput")
    ident = nc.dram_tensor("ident", [W, W], BF16, kind="ExternalInput")
    biaso = nc.dram_tensor("biaso", [O, 1], F32, kind="ExternalInput")
    out = nc.dram_tensor("out", [BS, O, H * W], F32, kind="ExternalOutput")

    def sb_view(tile, offset, dims):
        return bass.AP(tensor=tile.tensor, offset=int(tile.offset) + offset,
                       ap=[list(d) for d in dims])

    from contextlib import ExitStack
    with TileContext(nc) as tc:
        with ExitStack() as _stk:
            def _pool(*a, **kw):
                return _stk.enter_context(tc.tile_pool(*a, **kw))
            cpool = _pool(name="consts", bufs=1)
            xpool = _pool(name="xs", bufs=2)
            ytpool = _pool(name="yt", bufs=2)
            vpool = _pool(name="fvy", bufs=2)
            fpool = _pool(name="fscr", bufs=1)
            hbpool = _pool(name="hb", bufs=1)
            expool = _pool(name="ex", bufs=2)
            t0pool = _pool(name="t0", bufs=3)
            tmpool = _pool(name="tm", bufs=3)
            rpool = _pool(name="red", bufs=4)
            opool = _pool(name="obuf", bufs=2)
            zpool = _pool(name="zro", bufs=1)
            gpool = _pool(name="exg", bufs=1, space="DRAM")
            pyt = _pool(name="psum_yt", bufs=2, space="PSUM")
            pom = _pool(name="psum_om", bufs=1, space="PSUM")
            pxi = _pool(name="psum_xi", bufs=2, space="PSUM")
            pout = _pool(name="psum_o", bufs=1, space="PSUM")
            wmain_sb = cpool.tile([C, NTAP * O], BF16)
            womb_sb = cpool.tile([C, NTAP * 27], BF16)
            obrep_sb = cpool.tile([W, 27], F32)
            ity_sb = cpool.tile([W, NTY], F32)
            ident_sb = cpool.tile([W, W], BF16)
            biaso_sb = cpool.tile([O, 1], F32)
            nc.sync.dma_start(wmain_sb[:], wmain[:])
            nc.sync.dma_start(womb_sb[:], womb[:])
            nc.sync.dma_start(obrep_sb[:], obrep[:])
            nc.sync.dma_start(ity_sb[:], ity[:])
            nc.sync.dma_start(ident_sb[:], ident[:])
            nc.sync.dma_start(biaso_sb[:], biaso[:])

            # DRAM scratch for the dense Ex images (double buffered), plus
            # one-time zero fill (only the 7-band diagonals ever get
            # rewritten; everything else must read as zero).
            exgarr = [gpool.tile([BLKS, BLKSZ], BF16, name=f"exg{i}")
                      for i in range(2)]
            ztot = BLKS * BLKSZ
            zper = ztot // 128          # 9648
            zchunk = zper // 4          # 2412
            zsrc = zpool.tile([W, zchunk], BF16)
            nc.vector.memset(zsrc[:], 0.0)
            for g in exgarr:
                for q in range(4):
                    nc.sync.dma_start(
                        bass.AP(tensor=g.tensor,
                                offset=int(g.offset) + q * zchunk,
                                ap=[[zper, 128], [1, zchunk]]),
                        zsrc[:])

            # banded x-weights, partition-shifted: HB[s, (hh,k,uf)] holds the
            # Ex value for source col s, output col w = s - d, d = 3 - uf.
            # Strips that no shift writes stay zero forever.
            hb = hbpool.tile([W, FHB], BF16)
            nc.vector.memset(hb[:], 0.0)

            for img in range(BS):
                for band in range(nbands):
                    b0 = band * RB
                    exg = exgarr[(img * nbands + band) % 2]

                    # ---- x band in (padded rows [b0-2, b0+RB+4)) ----------
                    xs = xpool.tile([C, XBROW * Wp], BF16, tag="xs")
                    rlo = max(0, b0 - 2)
                    rhi = min(Hp, b0 + RB + 4)
                    dst0 = (rlo - (b0 - 2)) * Wp
                    nc.sync.dma_start(
                        xs[:, dst0:dst0 + (rhi - rlo) * Wp],
                        bass.AP(tensor=xp,
                                offset=img * C * Hp * Wp + rlo * Wp,
                                ap=[[Hp * Wp, C], [1, (rhi - rlo) * Wp]]))

                    # ---- stage 1: per-tap 1x1 convs into YT band ----------
                    yt = ytpool.tile([W, NROW * RS], BF16, tag="yt")
                    for rr in range(NROW):
                        r = b0 - 3 + rr
                        if r < 0 or r >= H:
                            nc.vector.memset(yt[:, rr * RS:(rr + 1) * RS], 0.0)
                            continue
                        lhsT = sb_view(xs, rr * Wp + 1, [[XBROW * Wp, C], [1, W]])
                        ps_y = pyt.tile([W, RS], F32, tag="y")
                        nc.tensor.matmul(ps_y[:, 0:512], lhsT,
                                         wmain_sb[:, 0:512],
                                         start=True, stop=True)
                        nc.tensor.matmul(ps_y[:, 512:RS], lhsT,
                                         wmain_sb[:, 512:RS],
                                         start=True, stop=True)
                        nc.scalar.copy(out=yt[:, rr * RS:(rr + 1) * RS],
                                       in_=ps_y[:])

                    # ---- offset/mask convs --------------------------------
                    raw = fpool.tile([W, RB * 27], F32, tag="raw")
                    for hh in range(RB):
                        ps_om = pom.tile([W, 27], F32, tag="om")
                        for t in range(NTAP):
                            ti, tj = t // 3, t % 3
                            lhsT = sb_view(xs, (hh + ti + 2) * Wp + tj,
                                           [[XBROW * Wp, C], [1, W]])
                            nc.tensor.matmul(ps_om[:], lhsT,
                                             womb_sb[:, t * 27:(t + 1) * 27],
                                             start=(t == 0), stop=(t == NTAP - 1))
                        nc.vector.tensor_add(
                            out=raw[:, hh * 27:(hh + 1) * 27],
                            in0=ps_om[:], in1=obrep_sb[:])

                    # ---- per-pixel interpolation fields (fp32 math) -------
                    dyv = sb_view(raw, 0, [[RB * 27, W], [27, RB], [2, NTAP]])
                    dxv = sb_view(raw, 1, [[RB * 27, W], [27, RB], [2, NTAP]])
                    mrawv = sb_view(raw, 18, [[RB * 27, W], [27, RB], [1, NTAP]])

                    def pk2(tile):  # packed [hh][k] view matching (RB, NTAP)
                        return sb_view(tile, 0, [[NK9, W], [NTAP, RB], [1, NTAP]])

                    msk = fpool.tile([W, NK9], F32, tag="msk")
                    nc.scalar.activation(out=pk2(msk), in_=mrawv, func=AF.Sigmoid)

                    MAGIC = 12582912.0  # 1.5 * 2**23: fp32 round-to-int magic

                    def frac_int(dv, tag):
                        # v = dv + 2 in (0.2, 3.8); e2 = floor(v); fr = v - e2
                        tt = fpool.tile([W, NK9], F32, tag=f"T{tag}")
                        t3 = fpool.tile([W, NK9], F32, tag=f"t3{tag}")
                        fr = fpool.tile([W, NK9], F32, tag=f"f{tag}")
                        e2 = fpool.tile([W, NK9], F32, tag=f"e{tag}")
                        nc.vector.tensor_scalar(out=pk2(tt), in0=dv, scalar1=2.0,
                                                scalar2=None, op0=AL.add)
                        nc.vector.tensor_scalar(out=t3[:], in0=tt[:],
                                                scalar1=-0.5, scalar2=MAGIC,
                                                op0=AL.add, op1=AL.add)
                        nc.vector.tensor_scalar(out=e2[:], in0=t3[:],
                                                scalar1=-MAGIC, scalar2=None,
                                                op0=AL.add)
                        nc.vector.tensor_sub(out=fr[:], in0=tt[:], in1=e2[:])
                        return fr, e2  # frac, floor+2 (exact int-valued)

                    fy, ey2 = frac_int(dyv, "y")
                    fx, ex2 = frac_int(dxv, "x")

                    def eq_pair(e2, tag):
                        # one-hots over the NTY candidate slots (c0 = floor+2)
                        c1 = fpool.tile([W, NK9], F32, tag=f"c1{tag}")
                        nc.vector.tensor_scalar(out=c1[:], in0=e2[:], scalar1=1.0,
                                                scalar2=None, op0=AL.add)
                        eq0 = fpool.tile([W, NK9 * NTY], F32, tag=f"eq0{tag}")
                        eq1 = fpool.tile([W, NK9 * NTY], F32, tag=f"eq1{tag}")
                        itv = sb_view(ity_sb, 0,
                                      [[NTY, W], [0, RB], [0, NTAP], [1, NTY]])
                        for eq, cc in ((eq0, e2), (eq1, c1)):
                            nc.vector.tensor_tensor(
                                out=sb_view(eq, 0, [[NK9 * NTY, W],
                                                    [NTAP * NTY, RB],
                                                    [NTY, NTAP], [1, NTY]]),
                                in0=itv,
                                in1=sb_view(cc, 0, [[NK9, W], [NTAP, RB],
                                                    [1, NTAP], [0, NTY]]),
                                op=AL.is_equal)
                        return eq0, eq1

                    eq0y, eq1y = eq_pair(ey2, "y")
                    eq0x, eq1x = eq_pair(ex2, "x")

                    def lerp(eq0, eq1, w1, w0, otile):
                        # otile = eq0*w0 + eq1*w1  ([W, NK9*NTY], bf16 out)
                        et = fpool.tile([W, NK9 * NTY], F32, tag="lerptmp")
                        res = fpool.tile([W, NK9 * NTY], F32, tag="lerpres")
                        bc = lambda t: sb_view(t, 0, [[NK9, W], [NTAP, RB],
                                                      [1, NTAP], [0, NTY]])
                        fl = lambda t: sb_view(t, 0, [[NK9 * NTY, W],
                                                      [NTAP * NTY, RB],
                                                      [NTY, NTAP], [1, NTY]])
                        nc.vector.tensor_tensor(out=fl(res), in0=fl(eq0),
                                                in1=bc(w0), op=AL.mult)
                        nc.vector.tensor_tensor(out=fl(et), in0=fl(eq1),
                                                in1=bc(w1), op=AL.mult)
                        nc.vector.tensor_add(out=otile[:], in0=res[:], in1=et[:])

                    fy1 = fpool.tile([W, NK9], F32, tag="fy1")
                    nc.vector.tensor_scalar(out=fy1[:], in0=fy[:], scalar1=-1.0,
                                            scalar2=1.0, op0=AL.mult, op1=AL.add)
                    vy = vpool.tile([W, FVY], BF16, tag="vy")
                    lerp(eq0y, eq1y, fy, fy1, vy)
                    fxm = fpool.tile([W, NK9], F32, tag="fxm")
                    fx1m = fpool.tile([W, NK9], F32, tag="fx1m")
                    nc.vector.tensor_mul(out=fxm[:], in0=fx[:], in1=msk[:])
                    nc.vector.tensor_sub(out=fx1m[:], in0=msk[:], in1=fxm[:])
                    hxm = vpool.tile([W, FVY], BF16, tag="hxm")
                    lerp(eq0x, eq1x, fxm, fx1m, hxm)

                    # ---- banded -> dense Ex via skewed DRAM round trip ----
                    # HB[s, hh, k, uf] = hxm[s-3+uf, hh, k, 6-uf-j], j = k%3
                    for uf in range(7):
                        jlo = max(0, 2 - uf)
                        jhi = min(2, 6 - uf)
                        nj = jhi - jlo + 1
                        delta = 3 - uf          # dst partition = src + delta
                        dlo, slo = max(0, delta), max(0, -delta)
                        cnt = W - abs(delta)
                        # (hh, i) merged: strides 63=3*21 / 45=3*15; one copy
                        # per j (the balancer appends a unit dim for
                        # non-contiguous innermost, and caps at 3 dims)
                        for j in range(jlo, jhi + 1):
                            nc.sync.dma_start(
                                sb_view(hb, dlo * FHB + j * 7 + uf,
                                        [[FHB, cnt], [21, 3 * RB]]),
                                sb_view(hxm, slo * FVY + j * 5 + (6 - uf - j),
                                        [[FVY, cnt], [15, 3 * RB]]))

                    # scatter HB -> dense DRAM image (write stride 135 vs
                    # read stride 134 skews the band onto block diagonals)
                    nc.sync.dma_start(
                        bass.AP(tensor=exg.tensor, offset=int(exg.offset),
                                ap=[[135, W], [BLKSZ, BLKS], [1, 7]]),
                        sb_view(hb, 0, [[FHB, W], [7, BLKS], [1, 7]]))
                    # read back dense: Ex[s, (hh, k, w)]
                    ex = expool.tile([W, FEX], BF16, tag="ex")
                    nc.sync.dma_start(
                        sb_view(ex, 0, [[FEX, W], [W, BLKS], [1, W]]),
                        bass.AP(tensor=exg.tensor, offset=int(exg.offset) + 3,
                                ap=[[ROWL, W], [BLKSZ, BLKS], [1, W]]))

                    # ---- combine ------------------------------------------
                    obuf = opool.tile([O, RB * W], F32, tag="obuf")
                    for hh in range(RB):
                        ps_o = pout.tile([O, W], F32, tag="po")
                        for k in range(NTAP):
                            i = k // 3
                            ps_x = pxi.tile([W, O * NTY], F32, tag="px")
                            lhsT = sb_view(ex, (hh * NTAP + k) * W,
                                           [[FEX, W], [1, W]])
                            rhs = sb_view(yt, (hh + i) * RS + k * O,
                                          [[NROW * RS, W], [1, O], [RS, NTY]])
                            nc.tensor.matmul(ps_x[:], lhsT, rhs,
                                             start=True, stop=True)

                            mode = TAPMODE[k]
                            vyv = sb_view(vy, hh * NTAP * NTY + k * NTY,
                                          [[FVY, W], [0, O], [1, NTY]])
                            red = rpool.tile([W, O], BF16, tag="red")
                            tm = tmpool.tile([W, O * NTY], BF16, tag="tm")
                            tmv = sb_view(tm, 0, [[O * NTY, W], [NTY, O], [1, NTY]])
                            if mode == "direct":
                                inv = sb_view(ps_x, 0, [[O * NTY, W],
                                                        [NTY, O], [1, NTY]])
                                nc.vector.tensor_tensor(out=tmv, in0=inv,
                                                        in1=vyv, op=AL.mult)
                            else:
                                t0 = t0pool.tile([W, O * NTY], BF16, tag="t0")
                                nc.scalar.copy(out=t0[:], in_=ps_x[:])
                                t0v = sb_view(t0, 0, [[O * NTY, W],
                                                      [NTY, O], [1, NTY]])
                                eng = (nc.gpsimd if mode == "act_pool"
                                       else nc.vector)
                                eng.tensor_tensor(out=tmv, in0=t0v,
                                                  in1=vyv, op=AL.mult)
                            with nc.allow_low_precision("bf16 tap partials"):
                                nc.vector.tensor_reduce(
                                    out=red[:],
                                    in_=sb_view(tm, 0, [[O * NTY, W],
                                                        [NTY, O], [1, NTY]]),
                                    axis=mybir.AxisListType.X, op=AL.add)
                            nc.tensor.matmul(ps_o[:], red[:], ident_sb[:],
                                             start=(k == 0), stop=(k == NTAP - 1))
                        nc.scalar.activation(out=obuf[:, hh * W:(hh + 1) * W],
                                             in_=ps_o[:], func=AF.Identity,
                                             bias=biaso_sb[:], scale=1.0)

                    nc.sync.dma_start(
                        bass.AP(tensor=out,
                                offset=img * O * H * W + b0 * W,
                                ap=[[H * W, O], [1, RB * W]]),
                        obuf[:])

    nc.compile()
    return nc


# ---------------------------------------------------------------------------
def _prep_host_inputs(x, weight, bias, offset_w, offset_b, mask_w, mask_b,
                      H, BS):
    """Build per-core input maps (host-side layout marshalling only)."""
    import ml_dtypes
    BF = ml_dtypes.bfloat16
    B = x.shape[0]
    Hp, Wp = H + 2, W + 2
    ncores = B // BS
    xp = np.zeros((B, C, Hp, Wp), BF)
    xp[:, :, 1:1 + H, 1:1 + W] = x.astype(BF)
    xp = xp.reshape(B, C, Hp * Wp)

    wmain = np.ascontiguousarray(
        weight.transpose(1, 2, 3, 0).reshape(C, NTAP * O)).astype(BF)
    wo = offset_w.transpose(1, 2, 3, 0)   # [C, 3, 3, 18]
    wm = mask_w.transpose(1, 2, 3, 0)     # [C, 3, 3, 9]
    womb = np.concatenate([wo, wm], axis=3).reshape(C, NTAP * 27)
    womb = np.ascontiguousarray(womb).astype(BF)
    ob27 = np.concatenate([offset_b, mask_b]).astype(np.float32)
    obrep = np.broadcast_to(ob27, (W, 27)).copy()
    ity = np.broadcast_to(np.arange(NTY, dtype=np.float32), (W, NTY)).copy()
    ident = np.eye(W, dtype=np.float32).astype(BF)
    biaso = bias.astype(np.float32).reshape(O, 1)

    shared = dict(wmain=wmain, womb=womb, obrep=obrep, ity=ity,
                  ident=ident, biaso=biaso)
    in_maps = []
    for corei in range(ncores):
        m = dict(shared)
        m["xp"] = np.ascontiguousarray(xp[corei * BS:(corei + 1) * BS])
        in_maps.append(m)
    return in_maps


_NC_CACHE = {}


def _get_nc(H=128, BS=2, RB=8):
    key = (H, BS, RB)
    if key not in _NC_CACHE:
        _NC_CACHE[key] = build_nc(H, BS, RB)
    return _NC_CACHE[key]


def kernel(x, weight, bias, offset_w, offset_b, mask_w, mask_b):
    from concourse.bass_utils import run_bass_kernel_spmd

    x = np.asarray(x, np.float32)
    B, _, H, _ = x.shape
    BS = B // NCORES
    nc = _get_nc(H=H, BS=BS)
    in_maps = _prep_host_inputs(
        x, np.asarray(weight), np.asarray(bias), np.asarray(offset_w),
        np.asarray(offset_b), np.asarray(mask_w), np.asarray(mask_b),
        H, BS)
    res = run_bass_kernel_spmd(nc, in_maps, core_ids=list(range(NCORES)))
    outs = [res.results[i]["out"].reshape(BS, O, H, W) for i in range(NCORES)]
    return np.concatenate(outs, axis=0)


# revision 26
# speedup vs baseline: 2.3446x; 2.3446x over previous
"""DCNv2 (deformable conv) Trainium2 Bass kernel, v2.

Strategy (per core, pure batch data-parallel across 8 cores):
  - x padded (+1) on host, cast bf16; streamed per band of RB=8 output rows.
  - PE computes offset/mask 3x3 convs (9 accumulating matmuls per output row)
    and per-tap 1x1 convs YT[s, r, k, o] = sum_c W[o,c,k] x(c,r,s) into a
    banded SBUF tensor (bf16), rows r with +-3 halo.
  - DVE builds per-pixel bilinear interpolation fields vy (y-weights) and
    hxm (x-weights with mask folded in), both [w, (hh, k, 5)] bf16.
  - The x-axis interpolation (column gather + lerp, a 7-banded per-(h,k)
    matrix Ex[s, w]) runs on PE: Ex is materialized densely via a DRAM
    round trip — hxm is partition-shifted into a band tensor HB[s,(hh,k,u)]
    (7 SBUF->SBUF DMA copies), HB is scattered to a DRAM image with a
    135-vs-134 skewed stride so the 7-wide band lands on the diagonals of
    dense 128x134 blocks (rest stays pre-zeroed), then read back dense.
    One matmul per (h, k): psum[w, (o, ty)] = Ex^T @ YT[:, rows(ty), k, o].
  - The y-axis interpolation is elementwise: multiply by vy (broadcast over
    o) and reduce over the 5 ty candidates -> red[w, o] bf16; spread across
    DVE / GpSimd / ACT per a static tap assignment.
  - Per-tap results accumulate on PE via transposing matmuls with an
    identity rhs: psum[o, w] += red_k^T; ACT adds bias on the PSUM->SBUF
    copy; DMA out per band.
"""

import sys

sys.path.insert(0, "/opt/trn_rl_repo")

import numpy as np

import concourse.bacc as bacc
import concourse.bass as bass
import concourse.mybir as mybir
from concourse.tile import TileContext

F32 = mybir.dt.float32
BF16 = mybir.dt.bfloat16
AF = mybir.ActivationFunctionType
AL = mybir.AluOpType

C = 96
O = 96
NTAP = 9
W = 128
NCORES = 8
NTY = 5                     # ty candidates (floor(dy) in [-2,1] + 1)
ROWL = 134                  # dense Ex row length (w' = w + 3, padded)
BLKSZ = 128 * ROWL          # 17152 elements per (hh,k) block

# per-tap engine assignment for the y-combine multiply:
#   "direct"   : DVE mult straight from PSUM (fp32 read)
#   "act_pool" : ACT copies PSUM->SBUF bf16, GpSimd mult
# (the ty-reduction always runs on DVE; GpSimd cannot reduce free axes)
TAPMODE = ["direct", "act_pool", "act_pool", "direct", "act_pool",
           "act_pool", "direct", "act_pool", "act_pool"]


def build_nc(H=128, BS=2, RB=8, num_devices=NCORES, debug_taps=False):
    RS = NTAP * O           # YT row stride = 864
    NROW = RB + 6           # YT band rows incl +-3 halo
    NK9 = RB * NTAP
    BLKS = RB * NTAP        # dense Ex blocks per band = 72
    Hp, Wp = H + 2, W + 2
    XBROW = RB + 6
    FVY = NK9 * NTY         # 360
    FHB = NK9 * 7           # 504
    FEX = NK9 * W           # 9216
    assert H % RB == 0
    nbands = H // RB

    nc = bacc.Bacc("TRN2", target_bir_lowering=False, debug=False,
                   num_devices=num_devices, dynamic_dma_scratch_size=10240)

    xp = nc.dram_tensor("xp", [BS, C, Hp * Wp], BF16, kind="ExternalInput")
    wmain = nc.dram_tensor("wmain", [C, NTAP * O], BF16, kind="ExternalInput")
    womb = nc.dram_tensor("womb", [C, NTAP * 27], BF16, kind="ExternalInput")
    obrep = nc.dram_tensor("obrep", [W, 27], F32, kind="ExternalInput")
    ity = nc.dram_tensor("ity", [W, NTY], F32, kind="ExternalInput")
    ident = nc.dram_tensor("ident", [W, W], BF16, kind="ExternalInput")
    eident = nc.dram_tensor("eident", [W, ROWL], BF16, kind="ExternalInput")
    biaso = nc.dram_tensor("biaso", [O, 1], F32, kind="ExternalInput")
    out = nc.dram_tensor("out", [BS, O, H * W], F32, kind="ExternalOutput")
    if debug_taps:
        d_raw = nc.dram_tensor("d_raw", [W, RB * 27], F32, kind="ExternalOutput")
        d_hxm = nc.dram_tensor("d_hxm", [W, RB * NTAP * NTY], BF16,
                               kind="ExternalOutput")
        d_vy = nc.dram_tensor("d_vy", [W, RB * NTAP * NTY], BF16,
                              kind="ExternalOutput")
        d_hb = nc.dram_tensor("d_hb", [W, RB * NTAP * 7], BF16,
                              kind="ExternalOutput")
        d_ex = nc.dram_tensor("d_ex", [W, RB * NTAP * ROWL], BF16,
                              kind="ExternalOutput")
        d_yt = nc.dram_tensor("d_yt", [W, (RB + 6) * NTAP * O], BF16,
                              kind="ExternalOutput")

    def sb_view(tile, offset, dims):
        return bass.AP(tensor=tile.tensor, offset=int(tile.offset) + offset,
                       ap=[list(d) for d in dims])

    from contextlib import ExitStack
    with TileContext(nc) as tc:
        with ExitStack() as _stk:
            def _pool(*a, **kw):
                return _stk.enter_context(tc.tile_pool(*a, **kw))
            cpool = _pool(name="consts", bufs=1)
            xpool = _pool(name="xs", bufs=2)
            ytpool = _pool(name="yt", bufs=2)
            vpool = _pool(name="fvy", bufs=2)
            fpool = _pool(name="fscr", bufs=1)
            hbpool = _pool(name="hb", bufs=1)
            expool = _pool(name="ex", bufs=2)
            t0pool = _pool(name="t0", bufs=3)
            tmpool = _pool(name="tm", bufs=3)
            rpool = _pool(name="red", bufs=4)
            opool = _pool(name="obuf", bufs=2)
            zpool = _pool(name="zro", bufs=1)
            gpool = _pool(name="exg", bufs=1, space="DRAM")
            pyt = _pool(name="psum_yt", bufs=2, space="PSUM")
            pom = _pool(name="psum_om", bufs=1, space="PSUM")
            phb = _pool(name="psum_hb", bufs=2, space="PSUM")
            pxi = _pool(name="psum_xi", bufs=2, space="PSUM")
            pout = _pool(name="psum_o", bufs=1, space="PSUM")
            wmain_sb = cpool.tile([C, NTAP * O], BF16)
            womb_sb = cpool.tile([C, NTAP * 27], BF16)
            obrep_sb = cpool.tile([W, 27], F32)
            ity_sb = cpool.tile([W, NTY], F32)
            ident_sb = cpool.tile([W, W], BF16)
            eident_sb = cpool.tile([W, ROWL], BF16)
            biaso_sb = cpool.tile([O, 1], F32)
            nc.sync.dma_start(wmain_sb[:], wmain[:])
            nc.sync.dma_start(womb_sb[:], womb[:])
            nc.sync.dma_start(obrep_sb[:], obrep[:])
            nc.sync.dma_start(ity_sb[:], ity[:])
            nc.sync.dma_start(ident_sb[:], ident[:])
            nc.sync.dma_start(eident_sb[:], eident[:])
            nc.sync.dma_start(biaso_sb[:], biaso[:])

            # DRAM scratch for the dense Ex images (double buffered), plus
            # one-time zero fill (only the 7-band diagonals ever get
            # rewritten; everything else must read as zero).
            exgarr = [gpool.tile([BLKS, BLKSZ], BF16, name=f"exg{i}")
                      for i in range(2)]
            ztot = BLKS * BLKSZ
            zper = ztot // 128          # 9648
            zchunk = zper // 4          # 2412
            zsrc = zpool.tile([W, zchunk], BF16)
            nc.vector.memset(zsrc[:], 0.0)
            for g in exgarr:
                for q in range(4):
                    nc.sync.dma_start(
                        bass.AP(tensor=g.tensor,
                                offset=int(g.offset) + q * zchunk,
                                ap=[[zper, 128], [1, zchunk]]),
                        zsrc[:])

            # banded x-weights, partition-shifted: HB[s, (hh,k,uf)] holds the
            # Ex value for source col s, output col w = s - d, d = 3 - uf.
            # Strips that no shift writes stay zero forever.
            hb = hbpool.tile([W, FHB], BF16)
            nc.vector.memset(hb[:], 0.0)

            for img in range(BS):
                for band in range(nbands):
                    b0 = band * RB
                    exg = exgarr[(img * nbands + band) % 2]

                    # ---- x band in (padded rows [b0-2, b0+RB+4)) ----------
                    xs = xpool.tile([C, XBROW * Wp], BF16, tag="xs")
                    rlo = max(0, b0 - 2)
                    rhi = min(Hp, b0 + RB + 4)
                    dst0 = (rlo - (b0 - 2)) * Wp
                    nc.sync.dma_start(
                        xs[:, dst0:dst0 + (rhi - rlo) * Wp],
                        bass.AP(tensor=xp,
                                offset=img * C * Hp * Wp + rlo * Wp,
                                ap=[[Hp * Wp, C], [1, (rhi - rlo) * Wp]]))

                    # ---- stage 1: per-tap 1x1 convs into YT band ----------
                    yt = ytpool.tile([W, NROW * RS], BF16, tag="yt")
                    for rr in range(NROW):
                        r = b0 - 3 + rr
                        if r < 0 or r >= H:
                            nc.vector.memset(yt[:, rr * RS:(rr + 1) * RS], 0.0)
                            continue
                        lhsT = sb_view(xs, rr * Wp + 1, [[XBROW * Wp, C], [1, W]])
                        for g in range(2):
                            ps_y = pyt.tile([W, RS // 2], F32, tag="y")
                            nc.tensor.matmul(ps_y[:], lhsT,
                                             wmain_sb[:, g * 432:(g + 1) * 432],
                                             start=True, stop=True)
                            nc.scalar.copy(
                                out=yt[:, rr * RS + g * 432:
                                       rr * RS + (g + 1) * 432],
                                in_=ps_y[:])

                    # ---- offset/mask convs --------------------------------
                    raw = fpool.tile([W, RB * 27], F32, tag="raw")
                    for hh in range(RB):
                        ps_om = pom.tile([W, 27], F32, tag="om")
                        for t in range(NTAP):
                            ti, tj = t // 3, t % 3
                            lhsT = sb_view(xs, (hh + ti + 2) * Wp + tj,
                                           [[XBROW * Wp, C], [1, W]])
                            nc.tensor.matmul(ps_om[:], lhsT,
                                             womb_sb[:, t * 27:(t + 1) * 27],
                                             start=(t == 0), stop=(t == NTAP - 1))
                        nc.vector.tensor_add(
                            out=raw[:, hh * 27:(hh + 1) * 27],
                            in0=ps_om[:], in1=obrep_sb[:])

                    # ---- per-pixel interpolation fields (fp32 math) -------
                    dyv = sb_view(raw, 0, [[RB * 27, W], [27, RB], [2, NTAP]])
                    dxv = sb_view(raw, 1, [[RB * 27, W], [27, RB], [2, NTAP]])
                    mrawv = sb_view(raw, 18, [[RB * 27, W], [27, RB], [1, NTAP]])

                    def pk2(tile):  # packed [hh][k] view matching (RB, NTAP)
                        return sb_view(tile, 0, [[NK9, W], [NTAP, RB], [1, NTAP]])

                    msk = fpool.tile([W, NK9], F32, tag="msk")
                    nc.scalar.activation(out=pk2(msk), in_=mrawv, func=AF.Sigmoid)

                    MAGIC = 12582912.0  # 1.5 * 2**23: fp32 round-to-int magic

                    def frac_int(dv, tag):
                        # v = dv + 2 in (0.2, 3.8); e2 = floor(v); fr = v - e2
                        tt = fpool.tile([W, NK9], F32, tag=f"T{tag}")
                        t3 = fpool.tile([W, NK9], F32, tag=f"t3{tag}")
                        fr = fpool.tile([W, NK9], F32, tag=f"f{tag}")
                        e2 = fpool.tile([W, NK9], F32, tag=f"e{tag}")
                        nc.vector.tensor_scalar(out=pk2(tt), in0=dv, scalar1=2.0,
                                                scalar2=None, op0=AL.add)
                        nc.vector.tensor_scalar(out=t3[:], in0=tt[:],
                                                scalar1=-0.5, scalar2=MAGIC,
                                                op0=AL.add, op1=AL.add)
                        nc.vector.tensor_scalar(out=e2[:], in0=t3[:],
                                                scalar1=-MAGIC, scalar2=None,
                                                op0=AL.add)
                        nc.vector.tensor_sub(out=fr[:], in0=tt[:], in1=e2[:])
                        return fr, e2  # frac, floor+2 (exact int-valued)

                    fy, ey2 = frac_int(dyv, "y")
                    fx, ex2 = frac_int(dxv, "x")

                    def eq_pair(e2, tag):
                        # one-hots over the NTY candidate slots (c0 = floor+2)
                        c1 = fpool.tile([W, NK9], F32, tag=f"c1{tag}")
                        nc.vector.tensor_scalar(out=c1[:], in0=e2[:], scalar1=1.0,
                                                scalar2=None, op0=AL.add)
                        eq0 = fpool.tile([W, NK9 * NTY], F32, tag=f"eq0{tag}")
                        eq1 = fpool.tile([W, NK9 * NTY], F32, tag=f"eq1{tag}")
                        itv = sb_view(ity_sb, 0,
                                      [[NTY, W], [0, RB], [0, NTAP], [1, NTY]])
                        for eq, cc in ((eq0, e2), (eq1, c1)):
                            nc.vector.tensor_tensor(
                                out=sb_view(eq, 0, [[NK9 * NTY, W],
                                                    [NTAP * NTY, RB],
                                                    [NTY, NTAP], [1, NTY]]),
                                in0=itv,
                                in1=sb_view(cc, 0, [[NK9, W], [NTAP, RB],
                                                    [1, NTAP], [0, NTY]]),
                                op=AL.is_equal)
                        return eq0, eq1

                    eq0y, eq1y = eq_pair(ey2, "y")
                    eq0x, eq1x = eq_pair(ex2, "x")

                    def lerp(eq0, eq1, w1, w0, otile):
                        # otile = eq0*w0 + eq1*w1  ([W, NK9*NTY], bf16 out)
                        et = fpool.tile([W, NK9 * NTY], F32, tag="lerptmp")
                        res = fpool.tile([W, NK9 * NTY], F32, tag="lerpres")
                        bc = lambda t: sb_view(t, 0, [[NK9, W], [NTAP, RB],
                                                      [1, NTAP], [0, NTY]])
                        fl = lambda t: sb_view(t, 0, [[NK9 * NTY, W],
                                                      [NTAP * NTY, RB],
                                                      [NTY, NTAP], [1, NTY]])
                        nc.vector.tensor_tensor(out=fl(res), in0=fl(eq0),
                                                in1=bc(w0), op=AL.mult)
                        nc.vector.tensor_tensor(out=fl(et), in0=fl(eq1),
                                                in1=bc(w1), op=AL.mult)
                        nc.vector.tensor_add(out=otile[:], in0=res[:], in1=et[:])

                    fy1 = fpool.tile([W, NK9], F32, tag="fy1")
                    nc.vector.tensor_scalar(out=fy1[:], in0=fy[:], scalar1=-1.0,
                                            scalar2=1.0, op0=AL.mult, op1=AL.add)
                    vy = vpool.tile([W, FVY], BF16, tag="vy")
                    lerp(eq0y, eq1y, fy, fy1, vy)
                    fxm = fpool.tile([W, NK9], F32, tag="fxm")
                    fx1m = fpool.tile([W, NK9], F32, tag="fx1m")
                    nc.vector.tensor_mul(out=fxm[:], in0=fx[:], in1=msk[:])
                    nc.vector.tensor_sub(out=fx1m[:], in0=msk[:], in1=fxm[:])
                    hxm = vpool.tile([W, FVY], BF16, tag="hxm")
                    lerp(eq0x, eq1x, fxm, fx1m, hxm)

                    # ---- banded -> dense Ex via skewed DRAM round trip ----
                    # HB[s, hh, k, uf] = hxm[s-3+uf, hh, k, 6-uf-j], j = k%3.
                    # The partition shifts run on PE: psum[s, .] = S_uf^T@hxm
                    # with S_uf a shifted-identity slice (uf-major blocks in
                    # psum, each uf's valid j-range contiguous), then ACT
                    # copies rearrange into HB's (blk, uf) layout.
                    for uf in range(7):
                        jlo = max(0, 2 - uf)
                        jhi = min(2, 6 - uf)
                        nj = jhi - jlo + 1
                        ps_hb = phb.tile([W, 72], F32, tag="hbb")
                        nc.tensor.matmul(
                            ps_hb[:, 0:24 * nj],
                            sb_view(eident_sb, uf, [[ROWL, W], [1, W]]),
                            sb_view(hxm, jlo * 4 + (6 - uf),
                                    [[FVY, W], [4, nj], [15, 3 * RB]]),
                            start=True, stop=True)
                        nc.scalar.copy(
                            out=sb_view(hb, jlo * 7 + uf,
                                        [[FHB, W], [7, nj], [21, 3 * RB]]),
                            in_=sb_view(ps_hb, 0,
                                        [[72, W], [24, nj], [1, 3 * RB]]))

                    # scatter HB -> dense DRAM image: write addr is
                    # s*9649 + blk*134 + uf (one extra element per
                    # partition), read rows are 9648 long, so the 7-band
                    # lands at w' = s + uf of row s; the rest stays zero.
                    nc.sync.dma_start(
                        bass.AP(tensor=exg.tensor, offset=int(exg.offset),
                                ap=[[BLKS * ROWL + 1, W], [ROWL, BLKS], [1, 7]]),
                        sb_view(hb, 0, [[FHB, W], [7, BLKS], [1, 7]]))
                    # read back dense (incl. pads): Ex[s, (blk, w')]
                    ex = expool.tile([W, BLKS * ROWL], BF16, tag="ex")
                    nc.sync.dma_start(
                        sb_view(ex, 0, [[BLKS * ROWL, W], [1, BLKS * ROWL]]),
                        bass.AP(tensor=exg.tensor, offset=int(exg.offset),
                                ap=[[BLKS * ROWL, W], [1, BLKS * ROWL]]))

                    if debug_taps and img == 0 and band == 0:
                        nc.sync.dma_start(d_raw[:], raw[:])
                        nc.sync.dma_start(d_hxm[:], hxm[:])
                        nc.sync.dma_start(d_vy[:], vy[:])
                        nc.sync.dma_start(d_hb[:], hb[:])
                        nc.sync.dma_start(d_ex[:], ex[:])
                        nc.sync.dma_start(d_yt[:], yt[:])

                    # ---- combine ------------------------------------------
                    obuf = opool.tile([O, RB * W], F32, tag="obuf")
                    for hh in range(RB):
                        ps_o = pout.tile([O, W], F32, tag="po")
                        for k in range(NTAP):
                            i = k // 3
                            ps_x = pxi.tile([W, O * NTY], F32, tag="px")
                            lhsT = sb_view(ex, (hh * NTAP + k) * ROWL + 3,
                                           [[BLKS * ROWL, W], [1, W]])
                            rhs = sb_view(yt, (hh + i) * RS + k * O,
                                          [[NROW * RS, W], [1, O], [RS, NTY]])
                            nc.tensor.matmul(ps_x[:], lhsT, rhs,
                                             start=True, stop=True)

                            mode = TAPMODE[k]
                            vyv = sb_view(vy, hh * NTAP * NTY + k * NTY,
                                          [[FVY, W], [0, O], [1, NTY]])
                            red = rpool.tile([W, O], BF16, tag="red")
                            tm = tmpool.tile([W, O * NTY], BF16, tag="tm")
                            tmv = sb_view(tm, 0, [[O * NTY, W], [NTY, O], [1, NTY]])
                            if mode == "direct":
                                inv = sb_view(ps_x, 0, [[O * NTY, W],
                                                        [NTY, O], [1, NTY]])
                                nc.vector.tensor_tensor(out=tmv, in0=inv,
                                                        in1=vyv, op=AL.mult)
                            else:
                                t0 = t0pool.tile([W, O * NTY], BF16, tag="t0")
                                nc.scalar.copy(out=t0[:], in_=ps_x[:])
                                t0v = sb_view(t0, 0, [[O * NTY, W],
                                                      [NTY, O], [1, NTY]])
                                eng = (nc.gpsimd if mode == "act_pool"
                                       else nc.vector)
                                eng.tensor_tensor(out=tmv, in0=t0v,
                                                  in1=vyv, op=AL.mult)
                            with nc.allow_low_precision("bf16 tap partials"):
                                nc.vector.tensor_reduce(
                                    out=red[:],
                                    in_=sb_view(tm, 0, [[O * NTY, W],
                                                        [NTY, O], [1, NTY]]),
                                    axis=mybir.AxisListType.X, op=AL.add)
                            nc.tensor.matmul(ps_o[:], red[:], ident_sb[:],
                                             start=(k == 0), stop=(k == NTAP - 1))
                        nc.scalar.activation(out=obuf[:, hh * W:(hh + 1) * W],
                                             in_=ps_o[:], func=AF.Identity,
                                             bias=biaso_sb[:], scale=1.0)

                    nc.sync.dma_start(
                        bass.AP(tensor=out,
                                offset=img * O * H * W + b0 * W,
                                ap=[[H * W, O], [1, RB * W]]),
                        obuf[:])

    nc.compile()
    return nc


# ---------------------------------------------------------------------------
def _prep_host_inputs(x, weight, bias, offset_w, offset_b, mask_w, mask_b,
                      H, BS):
    """Build per-core input maps (host-side layout marshalling only)."""
    import ml_dtypes
    BF = ml_dtypes.bfloat16
    B = x.shape[0]
    Hp, Wp = H + 2, W + 2
    ncores = B // BS
    xp = np.zeros((B, C, Hp, Wp), BF)
    xp[:, :, 1:1 + H, 1:1 + W] = x.astype(BF)
    xp = xp.reshape(B, C, Hp * Wp)

    wmain = np.ascontiguousarray(
        weight.transpose(1, 2, 3, 0).reshape(C, NTAP * O)).astype(BF)
    wo = offset_w.transpose(1, 2, 3, 0)   # [C, 3, 3, 18]
    wm = mask_w.transpose(1, 2, 3, 0)     # [C, 3, 3, 9]
    womb = np.concatenate([wo, wm], axis=3).reshape(C, NTAP * 27)
    womb = np.ascontiguousarray(womb).astype(BF)
    ob27 = np.concatenate([offset_b, mask_b]).astype(np.float32)
    obrep = np.broadcast_to(ob27, (W, 27)).copy()
    ity = np.broadcast_to(np.arange(NTY, dtype=np.float32), (W, NTY)).copy()
    ident = np.eye(W, dtype=np.float32).astype(BF)
    eident = np.zeros((W, ROWL), np.float32)
    eident[np.arange(W), np.arange(W) + 3] = 1.0
    eident = eident.astype(BF)
    biaso = bias.astype(np.float32).reshape(O, 1)

    shared = dict(wmain=wmain, womb=womb, obrep=obrep, ity=ity,
                  ident=ident, eident=eident, biaso=biaso)
    in_maps = []
    for corei in range(ncores):
        m = dict(shared)
        m["xp"] = np.ascontiguousarray(xp[corei * BS:(corei + 1) * BS])
        in_maps.append(m)
    return in_maps


_NC_CACHE = {}


def _get_nc(H=128, BS=2, RB=8):
    key = (H, BS, RB)
    if key not in _NC_CACHE:
        _NC_CACHE[key] = build_nc(H, BS, RB)
    return _NC_CACHE[key]


def kernel(x, weight, bias, offset_w, offset_b, mask_w, mask_b):
    from concourse.bass_utils import run_bass_kernel_spmd

    x = np.asarray(x, np.float32)
    B, _, H, _ = x.shape
    BS = B // NCORES
    nc = _get_nc(H=H, BS=BS)
    in_maps = _prep_host_inputs(
        x, np.asarray(weight), np.asarray(bias), np.asarray(offset_w),
        np.asarray(offset_b), np.asarray(mask_w), np.asarray(mask_b),
        H, BS)
    res = run_bass_kernel_spmd(nc, in_maps, core_ids=list(range(NCORES)))
    outs = [res.results[i]["out"].reshape(BS, O, H, W) for i in range(NCORES)]
    return np.concatenate(outs, axis=0)
